# revision 1
# baseline (speedup 1.0000x reference)
"""Mamba chunk-state kernel for Trainium2 (8 NeuronCores, Bass/Tile).

Computes, for inputs
    B  (b=4, s=8192, g=1, n=128)   f32
    x  (b=4, s=8192, h=32, p=64)   f32
    dt (b=4, h=32, c=32, l=256)    f32
    dA (b=4, h=32, c=32, l=256)    f32
the chunked state update
    states[b,c,h,p,n] = sum_l x[b,c,l,h,p] * scale[b,h,c,l] * B[b,c,l,n]
    scale = exp(dA[...,-1:] - dA) * dt

Sharding: core i handles batch b = i//2 and chunk range (i%2)*16..+16.
Each (b, chunk-range) slice is fully independent -> no collectives.

Per (b,c) chunk on a core:
  - x chunk [l=256, h*p=2048] loads naturally with l on partitions (two
    [128,2048] tiles); B chunk likewise ([128,128] x2). No transposes of
    the big tensors are ever needed.
  - scale is computed in its natural [h=32, l=256] layout (ACT exp with
    per-partition bias = dA_last, DVE multiply by dt), then the tiny
    [32,256] tile is PE-transposed to [l,h] so scale becomes a
    per-partition scalar for the x multiply.
  - xw = x * scale via 64 DVE tensor_scalar ops ([128,64] each, one per
    (head, l-half)).
  - states[2h*64+p, n] = sum_l xw[l, hp]  B[l, n]: 16 head-pairs x 2
    K-halves = 32 fp32 matmuls accumulating in PSUM ([128,512] bank
    tiles), ACT-copied to an SBUF staging tile, one DMA out per chunk.
"""

import numpy as np

BATCH, SEQLEN, NGROUPS, DSTATE = 4, 8192, 1, 128
NHEADS, HEADDIM, CHUNK = 32, 64, 256
NCHUNKS = SEQLEN // CHUNK  # 32
NCORES = 8
CPC = (BATCH * NCHUNKS) // NCORES  # 16 chunks per core
HP = NHEADS * HEADDIM  # 2048

_cached_nc = None


def _build_nc(repeat=1):
    import concourse.bacc as bacc
    import concourse.mybir as mybir
    import concourse.tile as tile
    from concourse.masks import make_identity

    f32 = mybir.dt.float32
    Exp = mybir.ActivationFunctionType.Exp

    nc = bacc.Bacc(
        "TRN2",
        target_bir_lowering=False,
        debug=False,
        num_devices=NCORES,
    )

    x_d = nc.dram_tensor("x_s", [CPC * CHUNK, HP], f32, kind="ExternalInput").ap()
    b_d = nc.dram_tensor("b_s", [CPC * CHUNK, DSTATE], f32, kind="ExternalInput").ap()
    dt_d = nc.dram_tensor("dt_s", [NHEADS, CPC * CHUNK], f32, kind="ExternalInput").ap()
    da_d = nc.dram_tensor("da_s", [NHEADS, CPC * CHUNK], f32, kind="ExternalInput").ap()
    out_d = nc.dram_tensor(
        "out_s", [CPC, HP, DSTATE], f32, kind="ExternalOutput"
    ).ap()

    with tile.TileContext(nc) as tc:
        with (
            tc.tile_pool(name="const", bufs=1) as const_pool,
            tc.tile_pool(name="meta", bufs=1) as meta_pool,
            tc.tile_pool(name="xin", bufs=8) as x_pool,
            tc.tile_pool(name="bin", bufs=4) as b_pool,
            tc.tile_pool(name="xwp", bufs=6) as xw_pool,
            tc.tile_pool(name="scp", bufs=3) as sc_pool,
            tc.tile_pool(name="stgp", bufs=4) as stg_pool,
            tc.tile_pool(name="pstates", bufs=6, space="PSUM") as ps_pool,
            tc.tile_pool(name="ptrans", bufs=2, space="PSUM") as pt_pool,
        ):
            ident = const_pool.tile([32, 32], f32)
            make_identity(nc, ident)

            # per-core dt / dA, loaded once: [32 heads, 16 chunks * 256]
            dt_t = meta_pool.tile([NHEADS, CPC * CHUNK], f32)
            da_t = meta_pool.tile([NHEADS, CPC * CHUNK], f32)
            nc.sync.dma_start(dt_t[:], dt_d[:])
            nc.sync.dma_start(da_t[:], da_d[:])

            for cc_rep in range(CPC * repeat):
                cc = cc_rep % CPC
                r0 = cc * CHUNK
                # ---- loads (l on partitions; fully contiguous rows) ----
                xh0 = x_pool.tile([128, HP], f32, name="xh0", tag="xh")
                xh1 = x_pool.tile([128, HP], f32, name="xh1", tag="xh")
                nc.sync.dma_start(xh0[:], x_d[r0 : r0 + 128, :])
                nc.sync.dma_start(xh1[:], x_d[r0 + 128 : r0 + 256, :])
                bh0 = b_pool.tile([128, DSTATE], f32, name="bh0", tag="bh")
                bh1 = b_pool.tile([128, DSTATE], f32, name="bh1", tag="bh")
                nc.sync.dma_start(bh0[:], b_d[r0 : r0 + 128, :])
                nc.sync.dma_start(bh1[:], b_d[r0 + 128 : r0 + 256, :])

                # ---- scale = exp(dA_last - dA) * dt, in [h, l] layout ----
                dec = sc_pool.tile([NHEADS, CHUNK], f32, name="dec", tag="dec")
                nc.scalar.activation(
                    dec[:],
                    da_t[:, r0 : r0 + CHUNK],
                    Exp,
                    bias=da_t[:, r0 + CHUNK - 1 : r0 + CHUNK],
                    scale=-1.0,
                )
                scl = sc_pool.tile([NHEADS, CHUNK], f32, name="scl", tag="scl")
                nc.vector.tensor_mul(scl[:], dec[:], dt_t[:, r0 : r0 + CHUNK])

                # ---- transpose scale to [l, h]: [32,256] -> [128, 64] ----
                # cols 0:32 = heads for l-half 0, cols 32:64 = l-half 1
                ptr = pt_pool.tile([128, 64], f32, name="ptr", tag="ptr")
                nc.tensor.transpose(ptr[:, 0:32], scl[:, 0:128], ident[:])
                nc.tensor.transpose(ptr[:, 32:64], scl[:, 128:256], ident[:])
                sct = sc_pool.tile([128, 64], f32, name="sct", tag="sct")
                nc.scalar.copy(sct[:], ptr[:])

                # ---- xw = x * scale (per-head per-partition scalar) ----
                xw0 = xw_pool.tile([128, HP], f32, name="xw0", tag="xw")
                xw1 = xw_pool.tile([128, HP], f32, name="xw1", tag="xw")
                for h in range(NHEADS):
                    c0 = h * HEADDIM
                    nc.vector.tensor_scalar_mul(
                        xw0[:, c0 : c0 + HEADDIM],
                        xh0[:, c0 : c0 + HEADDIM],
                        sct[:, h : h + 1],
                    )
                for h in range(NHEADS):
                    c0 = h * HEADDIM
                    nc.vector.tensor_scalar_mul(
                        xw1[:, c0 : c0 + HEADDIM],
                        xh1[:, c0 : c0 + HEADDIM],
                        sct[:, 32 + h : 32 + h + 1],
                    )

                # ---- states matmuls + PSUM -> SBUF -> DRAM ----
                stg = stg_pool.tile([128, HP], f32, name="stg", tag="stg")
                for q in range(4):
                    st = ps_pool.tile([128, 512], f32, name="st", tag="st")
                    for r in range(4):
                        hp = q * 4 + r
                        w0 = xw0[:, hp * 128 : (hp + 1) * 128]
                        w1 = xw1[:, hp * 128 : (hp + 1) * 128]
                        nc.tensor.matmul(
                            st[:, r * 128 : (r + 1) * 128], w0, bh0[:],
                            start=True, stop=False,
                        )
                        nc.tensor.matmul(
                            st[:, r * 128 : (r + 1) * 128], w1, bh1[:],
                            start=False, stop=True,
                        )
                    nc.scalar.copy(stg[:, q * 512 : (q + 1) * 512], st[:])

                # stg[dh*64+p, hp*128+n] -> out[(hp*2+dh)*64+p, n]
                out_ap = out_d[cc].rearrange(
                    "(hp dh p) n -> (dh p) hp n", hp=16, dh=2, p=HEADDIM
                )
                nc.scalar.dma_start(
                    out_ap, stg[:].rearrange("q (hp n) -> q hp n", hp=16)
                )

    nc.compile()
    return nc


def _get_nc():
    global _cached_nc
    if _cached_nc is None:
        _cached_nc = _build_nc()
    return _cached_nc


def _in_maps(B, x, dt, dA_cumsum):
    B = np.asarray(B, dtype=np.float32)
    x = np.asarray(x, dtype=np.float32)
    dt = np.asarray(dt, dtype=np.float32)
    dA = np.asarray(dA_cumsum, dtype=np.float32)
    maps = []
    for core in range(NCORES):
        b = core // 2
        c0 = (core % 2) * CPC
        s0, s1 = c0 * CHUNK, (c0 + CPC) * CHUNK
        maps.append(
            {
                "x_s": np.ascontiguousarray(x[b, s0:s1]).reshape(CPC * CHUNK, HP),
                "b_s": np.ascontiguousarray(B[b, s0:s1, 0, :]),
                "dt_s": np.ascontiguousarray(
                    dt[b, :, c0 : c0 + CPC, :]
                ).reshape(NHEADS, CPC * CHUNK),
                "da_s": np.ascontiguousarray(
                    dA[b, :, c0 : c0 + CPC, :]
                ).reshape(NHEADS, CPC * CHUNK),
            }
        )
    return maps


def _assemble(results):
    out = np.empty((BATCH, NCHUNKS, NHEADS, HEADDIM, DSTATE), np.float32)
    for core in range(NCORES):
        b = core // 2
        c0 = (core % 2) * CPC
        o = np.asarray(results[core]["out_s"])
        out[b, c0 : c0 + CPC] = o.reshape(CPC, NHEADS, HEADDIM, DSTATE)
    return out


def _run(B, x, dt, dA_cumsum, **run_kwargs):
    from concourse import bass_utils

    nc = _get_nc()
    res = bass_utils.run_bass_kernel_spmd(
        nc, _in_maps(B, x, dt, dA_cumsum), core_ids=list(range(NCORES)), **run_kwargs
    )
    return _assemble(res.results), res


def kernel(B, x, dt, dA_cumsum):
    out, _ = _run(B, x, dt, dA_cumsum)
    return out



# revision 2
# speedup vs baseline: 3.3559x; 3.3559x over previous
"""Mamba chunk-state kernel for Trainium2 (8 NeuronCores, Bass/Tile).

states[b,c,h,p,n] = sum_l x[b,c,l,h,p] * scale[b,h,c,l] * B[b,c,l,n]
scale = exp(dA[...,-1:] - dA) * dt

Memory-roofline design (the 8 cores share one chip's HBM, ~340 GB/s/core
effective; per-core traffic is 27.5 MB -> ~80 us floor):
  - bf16 for x / B / output (tolerance 2e-2; measured rel err ~6e-3):
    halves HBM traffic vs f32 and runs PE at 1 cycle/row (fp32 is 4).
  - x uploaded p-major ([s, p*32+h]) so the per-(h,l) scale multiply is a
    SINGLE DVE tensor_tensor per l-half with the small scale tile read
    through a stride-0 broadcast AP (innermost h stays packed -> 2x DVE
    perf mode), replacing 64 per-head tensor_scalar ops (~100 us DVE).
  - matmuls flipped: lhsT = B (m=128 dstate on PSUM partitions), rhs = xw
    (n=512 moving cols, one 4-bank PSUM tile per chunk) -> 8 bf16 matmuls
    per chunk; output leaves transposed [n, (p,h)] and the host
    untransposes during assembly.
  - scale computed in [l, h] layout directly (dt/dA uploaded as [s, h],
    dA_last replicated per chunk on host) -> no PE transpose, PSUM holds
    exactly two 4-bank chunk tiles (full double buffering).
  - every input is pre-arranged on host into its exact SBUF partition
    image and packed by dtype, so each 4-chunk superstep is 3 DMAs
    (x+B bf16 load, dt/dA/dal f32 load, store), all with >=4 KB
    contiguous descriptors (no <512B read-modify-write penalty).

Sharding: core i handles batch b = i//2 and chunk range (i%2)*16..+16.
Fully independent slices -> no collectives.
"""

import numpy as np
import ml_dtypes

BF16 = ml_dtypes.bfloat16

BATCH, SEQLEN, NGROUPS, DSTATE = 4, 8192, 1, 128
NHEADS, HEADDIM, CHUNK = 32, 64, 256
NCHUNKS = SEQLEN // CHUNK  # 32
NCORES = 8
CPC = (BATCH * NCHUNKS) // NCORES  # 16 chunks per core
HP = NHEADS * HEADDIM  # 2048
R = CPC * CHUNK  # 4096 rows per core
GROUP = 4  # chunks per superstep
NG = CPC // GROUP  # 4 supersteps

_cached_nc = None


def _build_nc(repeat=1, loop_trips=None, body_passes=4):
    import concourse.bacc as bacc
    import concourse.mybir as mybir
    import concourse.tile as tile

    f32 = mybir.dt.float32
    bf16 = mybir.dt.bfloat16
    Exp = mybir.ActivationFunctionType.Exp

    nc = bacc.Bacc(
        "TRN2",
        target_bir_lowering=False,
        debug=False,
        num_devices=NCORES,
    )

    G2 = GROUP * 2  # (chunk, half) blocks per superstep
    WXB = G2 * (HP + DSTATE)  # bf16 pack: x blocks then B blocks
    WMETA = G2 * NHEADS * 2 + GROUP * NHEADS  # f32 pack: dt, dA, dal
    xb_d = nc.dram_tensor("xb_s", [NG, 128, WXB], bf16, kind="ExternalInput").ap()
    mt_d = nc.dram_tensor("mt_s", [NG, 128, WMETA], f32, kind="ExternalInput").ap()
    out_d = nc.dram_tensor(
        "out_s", [NG, DSTATE, GROUP * HP], f32 if False else bf16, kind="ExternalOutput"
    ).ap()

    with tile.TileContext(nc) as tc:
        with (
            tc.tile_pool(name="xin", bufs=2) as x_pool,
            tc.tile_pool(name="bin", bufs=2) as b_pool,
            tc.tile_pool(name="meta", bufs=2) as m_pool,
            tc.tile_pool(name="scp", bufs=6) as sc_pool,
            tc.tile_pool(name="xwp", bufs=3) as xw_pool,
            tc.tile_pool(name="stgp", bufs=2) as stg_pool,
            tc.tile_pool(name="pstates", bufs=2, space="PSUM") as ps_pool,
        ):
            import contextlib

            if loop_trips is not None:
                loop_cm = tc.For_i(0, loop_trips)
                n_body = NG * body_passes
            else:
                loop_cm = contextlib.nullcontext()
                n_body = NG * repeat
            with loop_cm:
              for g_rep in range(n_body):
                g = g_rep % NG
                r0 = g * GROUP * CHUNK  # row offset in R-space
                # ---- superstep loads (one DMA per tensor) ----
                xbg = x_pool.tile([128, WXB], bf16, name="xbg", tag="xbg")
                nc.sync.dma_start(xbg[:], xb_d[g])
                mtg = m_pool.tile([128, WMETA], f32, name="mtg", tag="mtg")
                nc.sync.dma_start(mtg[:], mt_d[g])
                xg = xbg[:, : G2 * HP]
                bg = xbg[:, G2 * HP :]
                dtg = mtg[:, : G2 * NHEADS]
                dag = mtg[:, G2 * NHEADS : 2 * G2 * NHEADS]
                dalg = mtg[:, 2 * G2 * NHEADS :]

                stg = stg_pool.tile([128, GROUP * HP], bf16, name="stg", tag="stg")

                # ---- scale = exp(dA_last - dA) * dt for all chunks first,
                # so the tiny ACT exps don't queue behind big evict copies ----
                scts = []
                for k in range(GROUP):
                    diff = sc_pool.tile([128, 2 * NHEADS], f32, name="dif", tag="dif")
                    nc.gpsimd.tensor_sub(
                        diff.rearrange("l (f h) -> l f h", f=2),
                        dalg[:, k * NHEADS : (k + 1) * NHEADS]
                        .unsqueeze(1)
                        .broadcast_to((128, 2, NHEADS)),
                        dag[:, k * 2 * NHEADS : (k + 1) * 2 * NHEADS].rearrange(
                            "l (f h) -> l f h", f=2
                        ),
                    )
                    e = sc_pool.tile([128, 2 * NHEADS], f32, name="e", tag="e")
                    nc.scalar.activation(e[:], diff[:], Exp)
                    sct = sc_pool.tile([128, 2 * NHEADS], bf16, name="sct", tag="sct")
                    nc.gpsimd.tensor_mul(
                        sct[:], e[:], dtg[:, k * 2 * NHEADS : (k + 1) * 2 * NHEADS]
                    )
                    scts.append(sct)

                for k in range(GROUP):
                    # ---- xw = x * scale: ONE DVE op via broadcast AP ----
                    xw = xw_pool.tile([128, 2 * HP], bf16, name="xw", tag="xw")
                    nc.vector.tensor_mul(
                        xw.rearrange("l (f p h) -> l f p h", f=2, p=HEADDIM),
                        xg[:, k * 2 * HP : (k + 1) * 2 * HP].rearrange(
                            "l (f p h) -> l f p h", f=2, p=HEADDIM
                        ),
                        scts[k]
                        .rearrange("l (f h) -> l f h", f=2)
                        .unsqueeze(2)
                        .broadcast_to((128, 2, HEADDIM, NHEADS)),
                    )

                    # ---- states^T: lhsT=B (m=dstate), rhs=xw (n=512) ----
                    st = ps_pool.tile([128, HP], f32, name="st", tag="st")
                    for f in range(2):
                        for q in range(4):
                            nc.tensor.matmul(
                                st[:, q * 512 : (q + 1) * 512],
                                bg[:, (k * 2 + f) * DSTATE : (k * 2 + f + 1) * DSTATE],
                                xw[:, f * HP + q * 512 : f * HP + (q + 1) * 512],
                                start=(f == 0),
                                stop=(f == 1),
                            )
                    nc.scalar.copy(stg[:, k * HP : (k + 1) * HP], st[:])

                # ---- one store DMA per superstep ----
                nc.scalar.dma_start(out_d[g], stg[:])

    nc.compile()
    return nc


def _get_nc():
    global _cached_nc
    if _cached_nc is None:
        _cached_nc = _build_nc()
    return _cached_nc


def _in_maps(B, x, dt, dA_cumsum):
    B = np.asarray(B, dtype=np.float32)
    x = np.asarray(x, dtype=np.float32)
    dt = np.asarray(dt, dtype=np.float32)
    dA = np.asarray(dA_cumsum, dtype=np.float32)
    maps = []
    for core in range(NCORES):
        b = core // 2
        c0 = (core % 2) * CPC
        s0, s1 = c0 * CHUNK, (c0 + CPC) * CHUNK
        # x -> p-major [s, p*32+h], bf16
        xs = (
            np.ascontiguousarray(x[b, s0:s1].transpose(0, 2, 1))
            .reshape(R, HP)
            .astype(BF16)
        )
        bs = np.ascontiguousarray(B[b, s0:s1, 0, :]).astype(BF16)
        # dt/dA -> [s, h] f32
        dts = np.ascontiguousarray(
            dt[b, :, c0 : c0 + CPC, :].transpose(1, 2, 0)
        ).reshape(R, NHEADS)
        das = np.ascontiguousarray(
            dA[b, :, c0 : c0 + CPC, :].transpose(1, 2, 0)
        ).reshape(R, NHEADS)
        # dA_last replicated to [c*128+l, h] f32
        dal = np.repeat(
            np.ascontiguousarray(dA[b, :, c0 : c0 + CPC, -1].T), 128, axis=0
        )
        def pimg(arr, blocks, w):
            # [NG*blocks*128, w] -> [NG, 128, blocks*w] partition image
            return np.ascontiguousarray(
                arr.reshape(NG, blocks, 128, w).transpose(0, 2, 1, 3)
            ).reshape(NG, 128, blocks * w)

        maps.append(
            {
                "xb_s": np.concatenate(
                    [pimg(xs, GROUP * 2, HP), pimg(bs, GROUP * 2, DSTATE)], axis=2
                ),
                "mt_s": np.concatenate(
                    [
                        pimg(dts, GROUP * 2, NHEADS),
                        pimg(das, GROUP * 2, NHEADS),
                        pimg(dal, GROUP, NHEADS),
                    ],
                    axis=2,
                ),
            }
        )
    return maps


def _assemble(results):
    out = np.empty((BATCH, NCHUNKS, NHEADS, HEADDIM, DSTATE), np.float32)
    for core in range(NCORES):
        b = core // 2
        c0 = (core % 2) * CPC
        o = np.asarray(results[core]["out_s"]).astype(np.float32)
        # [NG, n, k*2048 + p*32 + h] -> [c, h, p, n]
        o = o.reshape(NG, DSTATE, GROUP, HEADDIM, NHEADS)
        out[b, c0 : c0 + CPC] = o.transpose(0, 2, 4, 3, 1).reshape(
            CPC, NHEADS, HEADDIM, DSTATE
        )
    return out


def _run(B, x, dt, dA_cumsum, **run_kwargs):
    from concourse import bass_utils

    nc = _get_nc()
    res = bass_utils.run_bass_kernel_spmd(
        nc, _in_maps(B, x, dt, dA_cumsum), core_ids=list(range(NCORES)), **run_kwargs
    )
    return _assemble(res.results), res


def kernel(B, x, dt, dA_cumsum):
    out, _ = _run(B, x, dt, dA_cumsum)
    return out
